# revision 25
# baseline (speedup 1.0000x reference)
"""Trainium2 Bass kernel for nn_ConvUnit (cimu bit-sliced int8 conv2d).

Reference computation:
  xq = int8(trunc(clip(x, -128, 127)))                    # [32,128,56,56]
  for i in 0..7:
    bit_i = (xq >> i) & 1                                  # {0,1}
    c_i   = conv2d_valid(bit_i, W)                         # [32,128,54,54]
    q_i   = clip(round_half_even(c_i / 2), -128, 127) * 2
    y    += q_i * (2^i  if i < 7 else -128)
  y += bias

Strategy (8 NeuronCores, data-parallel over batch, 4 images/core):
  * Weights host-prepped: W/2 split into bf16 hi+lo, each scaled by the
    per-plane factor k_i/2 (exact power-of-2 scaling), transposed to
    lhsT layout [ci, co].  PSUM then directly accumulates z = (c_i/2)*k_i
    where k_i = 2^(i+1) (i<7) / -256 (i=7).
  * Conv as 9 shifted matmuls (taps) x 2 (hi/lo) accumulating in PSUM,
    over flattened pixel windows; garbage columns (w>=54) discarded on
    output DMA.
  * round_half_even via the magic-constant trick: since clip never fires
    (checked on host: max_co sum|W|/2 << 127.5),
        u_i = RNE(z + M_i) - M_i  ==  k_i * round_half_even(c_i/2)
    with M_i = 1.5*2^23*|k_i|.  ACT does t = z + M_i (exact f32 add,
    HW-verified), DVE scalar_tensor_tensor fuses (t - M_i) + y.
  * Bit planes: exact trunc-toward-zero in f32 (abs/sign on ACT, magic
    round + is_gt fixup on DVE), convert to int32 (exact for integers),
    (xq >> i) & 1 on DVE, int32->bf16 convert on ACT.
"""
import sys

sys.path.insert(0, "/opt/trn_rl_repo")

import numpy as np
import ml_dtypes

import concourse.bass as bass
import concourse.tile as tile
from concourse import bacc, mybir
from concourse import bass_utils

BF16 = ml_dtypes.bfloat16

N_CORES = 8
B, C, H, W = 32, 128, 56, 56
HO, WO = 54, 54
BPC = B // N_CORES            # images per core
NPIX_IN = H * W               # 3136
NPIX = (HO - 1) * W + WO      # 3022 computed output positions / image
ROWS_PER_TILE = 9
TILE_N = ROWS_PER_TILE * W    # 504: 9 output rows x 56 (contiguous windows
                              # so DoubleRow rhs APs stay 3-D; 2 garbage
                              # cols/row are dropped on the output DMA)
TILES = [(j * TILE_N, min(TILE_N, NPIX - j * TILE_N))
         for j in range((NPIX + TILE_N - 1) // TILE_N)]   # 5x504 + 502
# Planes processed in DESCENDING k order: plane 7 first (its bit plane is
# just (x <= -1), so matmuls start early), and descending k lets the
# magic-rounding accumulate fuse into one DVE op per plane (y stays offset
# by the current magic M; y is always a multiple of the next plane's k).
PORDER = [7, 6, 5, 4, 3, 2, 1, 0]
# Per-plane weight representation (host flip-rate study, rel err 1.4e-2
# predicted vs 2e-2 budget):
#   f8    : e4m3 single level, taps DoubleRow-paired     (planes 0-1)
#   bf16  : single bf16 level                            (planes 2-4)
#   hi+f8 : bf16 hi + e4m3 lo(x2^9, rhs bits 2^-9), DR   (planes 5-7)
#   hilo  : bf16 hi + bf16 lo                            (unused)
REPR = ["f8", "f8", "bf16", "bf16", "bf16", "hi+f8", "hi+f8", "hi+f8"]
# DoubleRow tap pairs: (kx0,kx2) same row -> rhs pair stride 2; middle-col
# taps 1,4 -> stride 56; tap 7 runs as a plain (bf16-rate) fp8 matmul.
# (pair stride 1 is rejected by HW; 2/54/56 verified exact.)
DR_PAIRS = [(0, 2), (3, 5), (6, 8), (1, 4)]
DR_SINGLE = 7
N_BF_BLOCKS = {"hilo": 18, "bf16": 9, "hi+f8": 9, "f8": 0}
N_F8_BLOCKS = {"hilo": 0, "bf16": 0, "hi+f8": 9, "f8": 9}
WB_BF = sum(N_BF_BLOCKS[REPR[p]] for p in range(8))    # bf16 [128co] blocks
WB_F8 = sum(N_F8_BLOCKS[REPR[p]] for p in range(8))    # fp8  [128co] blocks
F8LO_SCALE = 512.0            # lo weights x2^9, rhs bits {0, 2^-9}

MAGIC = 12582912.0            # 1.5 * 2^23: RNE(z + MAGIC) - MAGIC == rhe(z)
# per-plane scale k_i applied to q (folded into weights as k_i/2)
KSCALE = [float(2 << i) for i in range(7)] + [-256.0]

AluOp = mybir.AluOpType
ActFn = mybir.ActivationFunctionType
F32 = mybir.dt.float32
I32 = mybir.dt.int32
BF = mybir.dt.bfloat16
F8 = mybir.dt.float8e4


E4M3 = ml_dtypes.float8_e4m3fn


def _prep_weights(weight: np.ndarray):
    """-> (bf16 [C, WB_BF*C], fp8 [C, WB_F8*C]) lhsT blocks, pre-scaled.

    bf16 blocks per plane (PORDER order): hilo -> 9 taps x (hi,lo);
    bf16/hi+f8 -> 9 hi taps.  fp8 blocks: f8 -> e4m3(w*2^p) per tap;
    hi+f8 -> e4m3(residual * 2^9) per tap; both in DoubleRow pair order
    [0,2, 3,5, 6,8, 1,4, 7].
    """
    wf = weight.astype(np.float32)
    dr_taps = [t for pr in DR_PAIRS for t in pr] + [DR_SINGLE]
    bf_blocks, f8_blocks = [], []
    for p in PORDER:
        kind = REPR[p]
        wk = wf * np.float32(0.5 * KSCALE[p])   # exact power-of-2 scale
        hi = wk.astype(BF16).astype(np.float32)
        if kind == "hilo":
            lo = (wk - hi).astype(BF16).astype(np.float32)
            for tap in range(9):
                kh, kw = tap // 3, tap % 3
                bf_blocks.append(hi[:, :, kh, kw].T.astype(BF16))
                bf_blocks.append(lo[:, :, kh, kw].T.astype(BF16))
        elif kind == "bf16":
            for tap in range(9):
                kh, kw = tap // 3, tap % 3
                bf_blocks.append(hi[:, :, kh, kw].T.astype(BF16))
        elif kind == "hi+f8":
            res = np.clip((wk - hi) * np.float32(F8LO_SCALE), -240, 240)
            for tap in range(9):
                kh, kw = tap // 3, tap % 3
                bf_blocks.append(hi[:, :, kh, kw].T.astype(BF16))
            for tap in dr_taps:
                kh, kw = tap // 3, tap % 3
                f8_blocks.append(res[:, :, kh, kw].T.astype(E4M3))
        else:  # f8 single
            wq = np.clip(wk, -240, 240)
            for tap in dr_taps:
                kh, kw = tap // 3, tap % 3
                f8_blocks.append(wq[:, :, kh, kw].T.astype(E4M3))
    wbf = np.ascontiguousarray(np.concatenate(
        [np.ascontiguousarray(b) for b in bf_blocks], axis=1))
    wf8 = np.ascontiguousarray(np.concatenate(
        [np.ascontiguousarray(b) for b in f8_blocks], axis=1))
    assert wbf.shape == (C, WB_BF * C) and wf8.shape == (C, WB_F8 * C)
    return wbf, wf8


def _build(need_clip: bool):
    nc = bacc.Bacc("TRN2", target_bir_lowering=False, debug=False,
                   num_devices=N_CORES)
    xs = nc.dram_tensor("xs", [BPC, C, NPIX_IN], F32, kind="ExternalInput").ap()
    wt = nc.dram_tensor("wt", [C, WB_BF * C], BF, kind="ExternalInput").ap()
    wt8 = nc.dram_tensor("wt8", [C, WB_F8 * C], F8, kind="ExternalInput").ap()
    bs = nc.dram_tensor("bs", [C, 1], F32, kind="ExternalInput").ap()
    out = nc.dram_tensor("out", [BPC, C, HO, WO], F32, kind="ExternalOutput").ap()

    with tile.TileContext(nc) as tc:
        with (
            tc.tile_pool(name="wpool", bufs=1) as wpool,
            tc.tile_pool(name="cpool", bufs=1) as cpool,
            tc.tile_pool(name="xpool", bufs=2) as xpool,
            tc.tile_pool(name="tpool", bufs=1) as tpool,
            tc.tile_pool(name="xqpool", bufs=2) as xqpool,
            tc.tile_pool(name="b32pool", bufs=2) as b32pool,
            tc.tile_pool(name="bitpool", bufs=3) as bitpool,
            tc.tile_pool(name="bit8pool", bufs=2) as bit8pool,
            tc.tile_pool(name="ypool", bufs=2) as ypool,
            tc.tile_pool(name="upool", bufs=6) as upool,
            tc.tile_pool(name="psum", bufs=8, space="PSUM") as pspool,
        ):
            wsb = wpool.tile([C, WB_BF * C], BF)
            # first processed plane's weights land first -> matmuls start early
            nc.sync.dma_start(wsb[:, :9 * C], wt[:, :9 * C])
            nc.sync.dma_start(wsb[:, 9 * C:], wt[:, 9 * C:])
            wsb8 = wpool.tile([C, WB_F8 * C], F8)
            nc.sync.dma_start(wsb8[:], wt8[:])
            bsb = cpool.tile([C, 1], F32)
            nc.sync.dma_start(bsb[:], bs[:])

            for img in range(BPC):
                xt = xpool.tile([C, NPIX_IN], F32, tag="x")
                nc.sync.dma_start(xt[:], xs[img])

                # ---- plane 7 bits straight from x: b7 = (x <= -1) ----
                bit7 = bitpool.tile([C, NPIX_IN], BF, tag="bit")
                nc.vector.tensor_scalar(bit7[:], xt[:], -1.0, None,
                                        AluOp.is_le)

                # ---- exact trunc-toward-zero: xq = trunc(clip(x)) ----
                # (hides behind plane-7 matmuls)
                # c = min(max(x, -128), 127)   (in place in xt)
                nc.vector.tensor_scalar(xt[:], xt[:], -128.0, 127.0,
                                        AluOp.max, AluOp.min)
                at = tpool.tile([C, NPIX_IN], F32, tag="ta")   # |c|
                nc.scalar.activation(at[:], xt[:], ActFn.Abs)
                st = tpool.tile([C, NPIX_IN], F32, tag="ts")   # sign(c)
                nc.scalar.activation(st[:], xt[:], ActFn.Sign)
                # f = rhe(|c|)   (reuse xt)
                nc.vector.tensor_scalar(xt[:], at[:], MAGIC, MAGIC,
                                        AluOp.add, AluOp.subtract)
                # g = (f > |c|)  (into at; at dead after)
                nc.vector.tensor_tensor(at[:], xt[:], at[:], AluOp.is_gt)
                # floor(|c|) = f - g   (into xt)
                nc.vector.tensor_tensor(xt[:], xt[:], at[:], AluOp.subtract)
                # trunc(c) = floor(|c|) * sign(c)  (into xt)
                nc.vector.tensor_tensor(xt[:], xt[:], st[:], AluOp.mult)
                # int32 convert (exact: integer-valued input)
                xq = xqpool.tile([C, NPIX_IN], I32, tag="xq")
                nc.vector.tensor_copy(xq[:], xt[:])

                yt = ypool.tile([C, HO * W], F32, tag="y")  # 3024, use 3022

                wbf0 = 0   # running bf16 / fp8 block cursors
                wf80 = 0
                for slot, plane in enumerate(PORDER):
                    kind = REPR[plane]
                    bit = bit8 = None
                    if plane == 7:
                        bit = bit7
                    else:
                        # ---- bit plane: ((xq >> plane) & 1) as bf16 ----
                        # (bitwise DVE ops cannot cast, so extract in i32
                        # then convert on ACT)
                        b32 = b32pool.tile([C, NPIX_IN], I32, tag="b32")
                        nc.vector.tensor_scalar(b32[:], xq[:], plane, 1,
                                                AluOp.logical_shift_right,
                                                AluOp.bitwise_and)
                        bit = bitpool.tile([C, NPIX_IN], BF, tag="bit")
                        nc.scalar.copy(bit[:], b32[:])
                    if kind == "f8":
                        # {0,1} in e4m3
                        bit8 = bit8pool.tile([C, NPIX_IN], F8, tag="bit8")
                        nc.scalar.copy(bit8[:], bit[:])
                    elif kind == "hi+f8":
                        # {0, 2^-9} in e4m3 (exact subnormal)
                        bit8 = bit8pool.tile([C, NPIX_IN], F8, tag="bit8")
                        nc.scalar.activation(bit8[:], bit[:], ActFn.Copy,
                                             scale=1.0 / F8LO_SCALE)

                    nbf = N_BF_BLOCKS[kind]
                    nf8 = N_F8_BLOCKS[kind]
                    # matmuls per (plane, tile): bf16 blocks stream singly;
                    # fp8 blocks stream as 4 DoubleRow pairs + 1 plain
                    nmm = nbf + (5 if nf8 else 0)
                    mag = MAGIC * abs(KSCALE[plane])
                    for j, (p0, nj) in enumerate(TILES):
                        ps = pspool.tile([C, TILE_N], F32, tag="ps")
                        imm = 0

                        def mm(lhsT, rhs, perf_mode=None):
                            nonlocal imm
                            nc.tensor.matmul(
                                ps[:, :nj], lhsT, rhs,
                                start=(imm == 0), stop=(imm == nmm - 1),
                                perf_mode=perf_mode)
                            imm += 1

                        def toff(tap):
                            return (tap // 3) * W + (tap % 3)

                        if kind == "hilo":
                            for tap in range(9):
                                o = p0 + toff(tap)
                                for half in range(2):
                                    widx = wbf0 + tap * 2 + half
                                    mm(wsb[:, widx * C:(widx + 1) * C],
                                       bit[:, o:o + nj])
                        elif kind in ("bf16", "hi+f8"):
                            for tap in range(9):
                                widx = wbf0 + tap
                                mm(wsb[:, widx * C:(widx + 1) * C],
                                   bit[:, p0 + toff(tap):p0 + toff(tap) + nj])
                        if nf8:
                            v = bit8[:]
                            for ip, (ta, tb) in enumerate(DR_PAIRS):
                                w0 = (wf80 + 2 * ip) * C
                                lhsT = wsb8[:, w0:w0 + 2 * C].rearrange(
                                    "p (two c) -> p two c", two=2)
                                oa, ob = p0 + toff(ta), p0 + toff(tb)
                                rhs = bass.AP(v.tensor, v.offset + oa,
                                              [list(v.ap[0]),
                                               [ob - oa, 2], [1, nj]])
                                mm(lhsT, rhs,
                                   perf_mode=mybir.MatmulPerfMode.DoubleRow)
                            w0 = (wf80 + 8) * C
                            o = p0 + toff(DR_SINGLE)
                            mm(wsb8[:, w0:w0 + C], bit8[:, o:o + nj])
                        assert imm == nmm
                        yv = yt[:, p0:p0 + nj]
                        # Fused magic-rounding accumulate (planes descend in
                        # k, so y is always an exact multiple of the current
                        # k and stays offset by the current magic M):
                        #   slot 0:  Y = fl(ps + M0)        = M0 + rhe(ps)
                        #   slot j:  Y = fl(fl(ps + Mj-Mj-1) + Y)
                        #              = Mj + y_prev + rhe(ps)   (all exact)
                        #   final :  y = (Y - M7s) + bias
                        # One DVE op per plane-tile, no ACT involvement.
                        if not need_clip:
                            if slot == 0:
                                nc.vector.tensor_scalar(yv, ps[:, :nj], mag,
                                                        None, AluOp.add)
                            else:
                                prev_mag = MAGIC * abs(KSCALE[PORDER[slot - 1]])
                                nc.vector.scalar_tensor_tensor(
                                    yv, ps[:, :nj], mag - prev_mag, yv,
                                    AluOp.add, AluOp.add)
                            if slot == 7:
                                nc.vector.tensor_scalar(yv, yv, mag,
                                                        bsb[:, 0:1],
                                                        AluOp.subtract,
                                                        AluOp.add)
                        else:
                            if slot == 0:
                                # y = rhe(psum) * k  directly from PSUM
                                nc.vector.tensor_scalar(yv, ps[:, :nj], mag,
                                                        mag, AluOp.add,
                                                        AluOp.subtract)
                            else:
                                # ACT: t = psum + M  (RNE -> mult of k)
                                ut = upool.tile([C, TILE_N], F32, tag="u")
                                nc.scalar.activation(ut[:, :nj], ps[:, :nj],
                                                     ActFn.Copy, bias=mag)
                                lok, hik = ((-128.0, 127.0)
                                            if KSCALE[plane] > 0
                                            else (-127.0, 128.0))
                                nc.vector.tensor_scalar(
                                    ut[:, :nj], ut[:, :nj],
                                    mag + lok * abs(KSCALE[plane]),
                                    mag + hik * abs(KSCALE[plane]),
                                    AluOp.max, AluOp.min)
                                # y = (t - M) + y   fused on DVE
                                nc.vector.scalar_tensor_tensor(
                                    yv, ut[:, :nj], mag, yv,
                                    AluOp.subtract, AluOp.add)
                            if slot == 7:
                                nc.vector.tensor_scalar(yv, yv, bsb[:, 0:1],
                                                        None, AluOp.add)
                        if slot == 7:
                            # per-tile writeout (9 output rows, 54 of 56 kept)
                            r0 = j * ROWS_PER_TILE
                            ysrc = yt[:].rearrange("p (h w) -> p h w", w=W)[
                                :, r0:r0 + ROWS_PER_TILE, 0:WO]
                            nc.sync.dma_start(
                                out[img][:, r0:r0 + ROWS_PER_TILE, :], ysrc)
                    wbf0 += nbf
                    wf80 += nf8

    nc.compile()
    return nc


_CACHE = {}


def _get_nc(need_clip: bool):
    if need_clip not in _CACHE:
        _CACHE[need_clip] = _build(need_clip)
    return _CACHE[need_clip]


def kernel(x: np.ndarray, weight: np.ndarray, bias: np.ndarray,
           _trace: bool = False):
    x = np.ascontiguousarray(x, dtype=np.float32)
    weight = np.ascontiguousarray(weight, dtype=np.float32)
    bias = np.ascontiguousarray(bias, dtype=np.float32)

    wbf_host, wf8_host = _prep_weights(weight)
    # clip in the reference only fires if |conv/2| can reach 127.5
    need_clip = float(np.abs(weight).sum(axis=(1, 2, 3)).max()) * 0.5 >= 127.4
    nc = _get_nc(need_clip)

    bs_host = bias.reshape(C, 1)
    xr = x.reshape(B, C, NPIX_IN)
    in_maps = []
    for c in range(N_CORES):
        in_maps.append({
            "xs": np.ascontiguousarray(xr[c * BPC:(c + 1) * BPC]),
            "wt": wbf_host,
            "wt8": wf8_host,
            "bs": bs_host,
        })

    res = bass_utils.run_bass_kernel_spmd(
        nc, in_maps, core_ids=list(range(N_CORES)), trace=_trace)

    y = np.concatenate([res.results[c]["out"] for c in range(N_CORES)], axis=0)
    if _trace:
        kernel._last_results = res
    return y


if __name__ == "__main__":
    np.random.seed(0)
    x = (np.random.randn(B, C, H, W) * 60).astype(np.float32)
    w = (np.random.randn(C, C, 3, 3) * 0.05).astype(np.float32)
    b = np.random.randn(C).astype(np.float32)
    y = kernel(x, w, b)
    print("out", y.shape, y.dtype)



# revision 27
# speedup vs baseline: 1.2002x; 1.2002x over previous
"""Trainium2 Bass kernel for nn_ConvUnit (cimu bit-sliced int8 conv2d).

Reference computation:
  xq = int8(trunc(clip(x, -128, 127)))                    # [32,128,56,56]
  for i in 0..7:
    bit_i = (xq >> i) & 1                                  # {0,1}
    c_i   = conv2d_valid(bit_i, W)                         # [32,128,54,54]
    q_i   = clip(round_half_even(c_i / 2), -128, 127) * 2
    y    += q_i * (2^i  if i < 7 else -128)
  y += bias

Strategy (8 NeuronCores, data-parallel over batch, 4 images/core):
  * Weights host-prepped: W/2 split into bf16 hi+lo, each scaled by the
    per-plane factor k_i/2 (exact power-of-2 scaling), transposed to
    lhsT layout [ci, co].  PSUM then directly accumulates z = (c_i/2)*k_i
    where k_i = 2^(i+1) (i<7) / -256 (i=7).
  * Conv as 9 shifted matmuls (taps) x 2 (hi/lo) accumulating in PSUM,
    over flattened pixel windows; garbage columns (w>=54) discarded on
    output DMA.
  * round_half_even via the magic-constant trick: since clip never fires
    (checked on host: max_co sum|W|/2 << 127.5),
        u_i = RNE(z + M_i) - M_i  ==  k_i * round_half_even(c_i/2)
    with M_i = 1.5*2^23*|k_i|.  ACT does t = z + M_i (exact f32 add,
    HW-verified), DVE scalar_tensor_tensor fuses (t - M_i) + y.
  * Bit planes: exact trunc-toward-zero in f32 (abs/sign on ACT, magic
    round + is_gt fixup on DVE), convert to int32 (exact for integers),
    (xq >> i) & 1 on DVE, int32->bf16 convert on ACT.
"""
import sys

sys.path.insert(0, "/opt/trn_rl_repo")

import numpy as np
import ml_dtypes

import concourse.bass as bass
import concourse.tile as tile
from concourse import bacc, mybir
from concourse import bass_utils

BF16 = ml_dtypes.bfloat16

N_CORES = 8
B, C, H, W = 32, 128, 56, 56
HO, WO = 54, 54
BPC = B // N_CORES            # images per core
NPIX_IN = H * W               # 3136
NPIX = (HO - 1) * W + WO      # 3022 computed output positions / image
ROWS_PER_TILE = 9
TILE_N = ROWS_PER_TILE * W    # 504: 9 output rows x 56 (contiguous windows
                              # so DoubleRow rhs APs stay 3-D; 2 garbage
                              # cols/row are dropped on the output DMA)
TILES = [(j * TILE_N, min(TILE_N, NPIX - j * TILE_N))
         for j in range((NPIX + TILE_N - 1) // TILE_N)]   # 5x504 + 502
# Planes processed in DESCENDING k order: plane 7 first (its bit plane is
# just (x <= -1), so matmuls start early), and descending k lets the
# magic-rounding accumulate fuse into one DVE op per plane (y stays offset
# by the current magic M; y is always a multiple of the next plane's k).
PORDER = [7, 6, 5, 4, 3, 2, 1, 0]
# Per-plane weight representation (host flip-rate study, rel err 1.4e-2
# predicted vs 2e-2 budget):
#   f8    : e4m3 single level, taps DoubleRow-paired     (planes 0-1)
#   bf16  : single bf16 level                            (planes 2-4)
#   hi+f8 : bf16 hi + e4m3 lo(x2^9, rhs bits 2^-9), DR   (planes 5-7)
#   hilo  : bf16 hi + bf16 lo                            (unused)
REPR = ["f8", "f8", "bf16", "bf16", "bf16", "hi+f8", "hi+f8", "hi+f8"]
# DoubleRow tap pairs: (kx0,kx2) same row -> rhs pair stride 2; middle-col
# taps 1,4 -> stride 56; tap 7 runs as a plain (bf16-rate) fp8 matmul.
# (pair stride 1 is rejected by HW; 2/54/56 verified exact.)
DR_PAIRS = [(0, 2), (3, 5), (6, 8), (1, 4)]
DR_SINGLE = 7
N_BF_BLOCKS = {"hilo": 18, "bf16": 9, "hi+f8": 9, "f8": 0}
N_F8_BLOCKS = {"hilo": 0, "bf16": 0, "hi+f8": 9, "f8": 9}
WB_BF = sum(N_BF_BLOCKS[REPR[p]] for p in range(8))    # bf16 [128co] blocks
WB_F8 = sum(N_F8_BLOCKS[REPR[p]] for p in range(8))    # fp8  [128co] blocks
F8LO_SCALE = 512.0            # lo weights x2^9, rhs bits {0, 2^-9}

MAGIC = 12582912.0            # 1.5 * 2^23: RNE(z + MAGIC) - MAGIC == rhe(z)
# per-plane scale k_i applied to q (folded into weights as k_i/2)
KSCALE = [float(2 << i) for i in range(7)] + [-256.0]

AluOp = mybir.AluOpType
ActFn = mybir.ActivationFunctionType
F32 = mybir.dt.float32
I32 = mybir.dt.int32
BF = mybir.dt.bfloat16
F8 = mybir.dt.float8e4


E4M3 = ml_dtypes.float8_e4m3fn


def _prep_weights(weight: np.ndarray):
    """-> (bf16 [C, WB_BF*C], fp8 [C, WB_F8*C]) lhsT blocks, pre-scaled.

    bf16 blocks per plane (PORDER order): hilo -> 9 taps x (hi,lo);
    bf16/hi+f8 -> 9 hi taps.  fp8 blocks: f8 -> e4m3(w*2^p) per tap;
    hi+f8 -> e4m3(residual * 2^9) per tap; both in DoubleRow pair order
    [0,2, 3,5, 6,8, 1,4, 7].
    """
    wf = weight.astype(np.float32)
    dr_taps = [t for pr in DR_PAIRS for t in pr] + [DR_SINGLE]
    bf_blocks, f8_blocks = [], []
    for p in PORDER:
        kind = REPR[p]
        wk = wf * np.float32(0.5 * KSCALE[p])   # exact power-of-2 scale
        hi = wk.astype(BF16).astype(np.float32)
        if kind == "hilo":
            lo = (wk - hi).astype(BF16).astype(np.float32)
            for tap in range(9):
                kh, kw = tap // 3, tap % 3
                bf_blocks.append(hi[:, :, kh, kw].T.astype(BF16))
                bf_blocks.append(lo[:, :, kh, kw].T.astype(BF16))
        elif kind == "bf16":
            for tap in range(9):
                kh, kw = tap // 3, tap % 3
                bf_blocks.append(hi[:, :, kh, kw].T.astype(BF16))
        elif kind == "hi+f8":
            res = np.clip((wk - hi) * np.float32(F8LO_SCALE), -240, 240)
            for tap in range(9):
                kh, kw = tap // 3, tap % 3
                bf_blocks.append(hi[:, :, kh, kw].T.astype(BF16))
            for tap in dr_taps:
                kh, kw = tap // 3, tap % 3
                f8_blocks.append(res[:, :, kh, kw].T.astype(E4M3))
        else:  # f8 single
            wq = np.clip(wk, -240, 240)
            for tap in dr_taps:
                kh, kw = tap // 3, tap % 3
                f8_blocks.append(wq[:, :, kh, kw].T.astype(E4M3))
    wbf = np.ascontiguousarray(np.concatenate(
        [np.ascontiguousarray(b) for b in bf_blocks], axis=1))
    wf8 = np.ascontiguousarray(np.concatenate(
        [np.ascontiguousarray(b) for b in f8_blocks], axis=1))
    assert wbf.shape == (C, WB_BF * C) and wf8.shape == (C, WB_F8 * C)
    return wbf, wf8


def _build(need_clip: bool):
    nc = bacc.Bacc("TRN2", target_bir_lowering=False, debug=False,
                   num_devices=N_CORES)
    xs = nc.dram_tensor("xs", [BPC, C, NPIX_IN], F32, kind="ExternalInput").ap()
    wt = nc.dram_tensor("wt", [C, WB_BF * C], BF, kind="ExternalInput").ap()
    wt8 = nc.dram_tensor("wt8", [C, WB_F8 * C], F8, kind="ExternalInput").ap()
    bs = nc.dram_tensor("bs", [C, 1], F32, kind="ExternalInput").ap()
    out = nc.dram_tensor("out", [BPC, C, HO, WO], F32, kind="ExternalOutput").ap()

    with tile.TileContext(nc) as tc:
        with (
            tc.tile_pool(name="wpool", bufs=1) as wpool,
            tc.tile_pool(name="cpool", bufs=1) as cpool,
            tc.tile_pool(name="xpool", bufs=2) as xpool,
            tc.tile_pool(name="tpool", bufs=1) as tpool,
            tc.tile_pool(name="xqpool", bufs=2) as xqpool,
            tc.tile_pool(name="b32pool", bufs=2) as b32pool,
            tc.tile_pool(name="bitpool", bufs=3) as bitpool,
            tc.tile_pool(name="bit8pool", bufs=2) as bit8pool,
            tc.tile_pool(name="ypool", bufs=2) as ypool,
            tc.tile_pool(name="upool", bufs=6) as upool,
            tc.tile_pool(name="psum", bufs=8, space="PSUM") as pspool,
        ):
            wsb = wpool.tile([C, WB_BF * C], BF)
            # first processed plane's weights land first -> matmuls start early
            nc.sync.dma_start(wsb[:, :9 * C], wt[:, :9 * C])
            nc.sync.dma_start(wsb[:, 9 * C:], wt[:, 9 * C:])
            wsb8 = wpool.tile([C, WB_F8 * C], F8)
            nc.sync.dma_start(wsb8[:], wt8[:])
            bsb = cpool.tile([C, 1], F32)
            nc.sync.dma_start(bsb[:], bs[:])

            for img in range(BPC):
                xt = xpool.tile([C, NPIX_IN], F32, tag="x")
                nc.sync.dma_start(xt[:], xs[img])

                def emit_bits(plane, xq):
                    """Materialize a plane's bit tensors (bf16 [+ e4m3])."""
                    kind = REPR[plane]
                    bit = bitpool.tile([C, NPIX_IN], BF, tag="bit")
                    if plane == 7:
                        # plane 7 straight from x: b7 = (x <= -1)
                        nc.vector.tensor_scalar(bit[:], xt[:], -1.0, None,
                                                AluOp.is_le)
                    else:
                        # ((xq >> plane) & 1): bitwise DVE ops cannot cast,
                        # so extract in i32 then convert on ACT
                        b32 = b32pool.tile([C, NPIX_IN], I32, tag="b32")
                        nc.vector.tensor_scalar(b32[:], xq[:], plane, 1,
                                                AluOp.logical_shift_right,
                                                AluOp.bitwise_and)
                        nc.scalar.copy(bit[:], b32[:])
                    bit8 = None
                    if kind == "f8":
                        # {0,1} in e4m3
                        bit8 = bit8pool.tile([C, NPIX_IN], F8, tag="bit8")
                        nc.scalar.copy(bit8[:], bit[:])
                    elif kind == "hi+f8":
                        # {0, 2^-9} in e4m3 (exact subnormal)
                        bit8 = bit8pool.tile([C, NPIX_IN], F8, tag="bit8")
                        nc.scalar.activation(bit8[:], bit[:], ActFn.Copy,
                                             scale=1.0 / F8LO_SCALE)
                    return bit, bit8

                # first plane's bits need no trunc ladder -> matmuls start
                # immediately; the ladder hides behind them
                bits = {PORDER[0]: emit_bits(PORDER[0], None)}

                # ---- exact trunc-toward-zero: xq = trunc(clip(x)) ----
                # c = min(max(x, -128), 127)   (in place in xt)
                nc.vector.tensor_scalar(xt[:], xt[:], -128.0, 127.0,
                                        AluOp.max, AluOp.min)
                at = tpool.tile([C, NPIX_IN], F32, tag="ta")   # |c|
                nc.scalar.activation(at[:], xt[:], ActFn.Abs)
                st = tpool.tile([C, NPIX_IN], F32, tag="ts")   # sign(c)
                nc.scalar.activation(st[:], xt[:], ActFn.Sign)
                # f = rhe(|c|)   (reuse xt)
                nc.vector.tensor_scalar(xt[:], at[:], MAGIC, MAGIC,
                                        AluOp.add, AluOp.subtract)
                # g = (f > |c|)  (into at; at dead after)
                nc.vector.tensor_tensor(at[:], xt[:], at[:], AluOp.is_gt)
                # floor(|c|) = f - g   (into xt)
                nc.vector.tensor_tensor(xt[:], xt[:], at[:], AluOp.subtract)
                # trunc(c) = floor(|c|) * sign(c)  (into xt)
                nc.vector.tensor_tensor(xt[:], xt[:], st[:], AluOp.mult)
                # int32 convert (exact: integer-valued input)
                xq = xqpool.tile([C, NPIX_IN], I32, tag="xq")
                nc.vector.tensor_copy(xq[:], xt[:])

                yt = ypool.tile([C, HO * W], F32, tag="y")  # 3024, use 3022

                wbf0 = 0   # running bf16 / fp8 block cursors
                wf80 = 0
                for slot, plane in enumerate(PORDER):
                    kind = REPR[plane]
                    # prefetch next plane's bits so its ACT conversions sit
                    # ahead of this plane's psum-retire ops in the ACT FIFO
                    if slot + 1 < len(PORDER):
                        nxt = PORDER[slot + 1]
                        bits[nxt] = emit_bits(nxt, xq)
                    bit, bit8 = bits.pop(plane)

                    nbf = N_BF_BLOCKS[kind]
                    nf8 = N_F8_BLOCKS[kind]
                    # matmuls per (plane, tile): bf16 blocks stream singly;
                    # fp8 blocks stream as 4 DoubleRow pairs + 1 plain
                    nmm = nbf + (5 if nf8 else 0)
                    mag = MAGIC * abs(KSCALE[plane])
                    for j, (p0, nj) in enumerate(TILES):
                        ps = pspool.tile([C, TILE_N], F32, tag="ps")
                        imm = 0

                        def mm(lhsT, rhs, perf_mode=None):
                            nonlocal imm
                            nc.tensor.matmul(
                                ps[:, :nj], lhsT, rhs,
                                start=(imm == 0), stop=(imm == nmm - 1),
                                perf_mode=perf_mode)
                            imm += 1

                        def toff(tap):
                            return (tap // 3) * W + (tap % 3)

                        if kind == "hilo":
                            for tap in range(9):
                                o = p0 + toff(tap)
                                for half in range(2):
                                    widx = wbf0 + tap * 2 + half
                                    mm(wsb[:, widx * C:(widx + 1) * C],
                                       bit[:, o:o + nj])
                        elif kind in ("bf16", "hi+f8"):
                            for tap in range(9):
                                widx = wbf0 + tap
                                mm(wsb[:, widx * C:(widx + 1) * C],
                                   bit[:, p0 + toff(tap):p0 + toff(tap) + nj])
                        if nf8:
                            v = bit8[:]
                            for ip, (ta, tb) in enumerate(DR_PAIRS):
                                w0 = (wf80 + 2 * ip) * C
                                lhsT = wsb8[:, w0:w0 + 2 * C].rearrange(
                                    "p (two c) -> p two c", two=2)
                                oa, ob = p0 + toff(ta), p0 + toff(tb)
                                rhs = bass.AP(v.tensor, v.offset + oa,
                                              [list(v.ap[0]),
                                               [ob - oa, 2], [1, nj]])
                                mm(lhsT, rhs,
                                   perf_mode=mybir.MatmulPerfMode.DoubleRow)
                            w0 = (wf80 + 8) * C
                            o = p0 + toff(DR_SINGLE)
                            mm(wsb8[:, w0:w0 + C], bit8[:, o:o + nj])
                        assert imm == nmm
                        yv = yt[:, p0:p0 + nj]
                        # Magic-rounding accumulate, split across engines so
                        # the ACT op retires the PSUM bank immediately and
                        # the DVE chain never gates the PE (planes descend
                        # in k, so y is an exact multiple of the current k
                        # and stays offset by the current magic M):
                        #   ACT :  u_j = fl(ps + Mj)       = Mj + rhe(ps)
                        #   DVE :  Y_j = fl((u_j - Mj-1) + Y_{j-1})
                        #             = Mj + y_prev + rhe(ps)    (all exact)
                        #   final: y = (Y - M7s) + bias
                        if not need_clip:
                            if slot == 0:
                                # u_0 is Y_0; ACT writes it straight into y
                                nc.scalar.activation(yv, ps[:, :nj],
                                                     ActFn.Copy, bias=mag)
                            else:
                                prev_mag = MAGIC * abs(KSCALE[PORDER[slot - 1]])
                                ut = upool.tile([C, TILE_N], F32, tag="u")
                                nc.scalar.activation(ut[:, :nj], ps[:, :nj],
                                                     ActFn.Copy, bias=mag)
                                nc.vector.scalar_tensor_tensor(
                                    yv, ut[:, :nj], prev_mag, yv,
                                    AluOp.subtract, AluOp.add)
                            if slot == 7:
                                nc.vector.tensor_scalar(yv, yv, mag,
                                                        bsb[:, 0:1],
                                                        AluOp.subtract,
                                                        AluOp.add)
                        else:
                            if slot == 0:
                                # y = rhe(psum) * k  directly from PSUM
                                nc.vector.tensor_scalar(yv, ps[:, :nj], mag,
                                                        mag, AluOp.add,
                                                        AluOp.subtract)
                            else:
                                # ACT: t = psum + M  (RNE -> mult of k)
                                ut = upool.tile([C, TILE_N], F32, tag="u")
                                nc.scalar.activation(ut[:, :nj], ps[:, :nj],
                                                     ActFn.Copy, bias=mag)
                                lok, hik = ((-128.0, 127.0)
                                            if KSCALE[plane] > 0
                                            else (-127.0, 128.0))
                                nc.vector.tensor_scalar(
                                    ut[:, :nj], ut[:, :nj],
                                    mag + lok * abs(KSCALE[plane]),
                                    mag + hik * abs(KSCALE[plane]),
                                    AluOp.max, AluOp.min)
                                # y = (t - M) + y   fused on DVE
                                nc.vector.scalar_tensor_tensor(
                                    yv, ut[:, :nj], mag, yv,
                                    AluOp.subtract, AluOp.add)
                            if slot == 7:
                                nc.vector.tensor_scalar(yv, yv, bsb[:, 0:1],
                                                        None, AluOp.add)
                        if slot == 7:
                            # per-tile writeout (9 output rows, 54 of 56 kept)
                            r0 = j * ROWS_PER_TILE
                            ysrc = yt[:].rearrange("p (h w) -> p h w", w=W)[
                                :, r0:r0 + ROWS_PER_TILE, 0:WO]
                            nc.sync.dma_start(
                                out[img][:, r0:r0 + ROWS_PER_TILE, :], ysrc)
                    wbf0 += nbf
                    wf80 += nf8

    nc.compile()
    return nc


_CACHE = {}


def _get_nc(need_clip: bool):
    if need_clip not in _CACHE:
        _CACHE[need_clip] = _build(need_clip)
    return _CACHE[need_clip]


def kernel(x: np.ndarray, weight: np.ndarray, bias: np.ndarray,
           _trace: bool = False):
    x = np.ascontiguousarray(x, dtype=np.float32)
    weight = np.ascontiguousarray(weight, dtype=np.float32)
    bias = np.ascontiguousarray(bias, dtype=np.float32)

    wbf_host, wf8_host = _prep_weights(weight)
    # clip in the reference only fires if |conv/2| can reach 127.5
    need_clip = float(np.abs(weight).sum(axis=(1, 2, 3)).max()) * 0.5 >= 127.4
    nc = _get_nc(need_clip)

    bs_host = bias.reshape(C, 1)
    xr = x.reshape(B, C, NPIX_IN)
    in_maps = []
    for c in range(N_CORES):
        in_maps.append({
            "xs": np.ascontiguousarray(xr[c * BPC:(c + 1) * BPC]),
            "wt": wbf_host,
            "wt8": wf8_host,
            "bs": bs_host,
        })

    res = bass_utils.run_bass_kernel_spmd(
        nc, in_maps, core_ids=list(range(N_CORES)), trace=_trace)

    y = np.concatenate([res.results[c]["out"] for c in range(N_CORES)], axis=0)
    if _trace:
        kernel._last_results = res
    return y


if __name__ == "__main__":
    np.random.seed(0)
    x = (np.random.randn(B, C, H, W) * 60).astype(np.float32)
    w = (np.random.randn(C, C, 3, 3) * 0.05).astype(np.float32)
    b = np.random.randn(C).astype(np.float32)
    y = kernel(x, w, b)
    print("out", y.shape, y.dtype)

